# revision 15
# baseline (speedup 1.0000x reference)
"""GraphGym GeneralConv (GCN-style, add-aggr, symmetric norm) on 8 Trainium2
NeuronCores via Bass/Tile.

Math (matches the reference exactly, up to fp reassociation):
    deg[i]  = 1 + #{e : row[e] == i}
    dis     = deg ** -0.5
    hn      = (x @ W) * dis[:, None]            # dis premultiplied, bf16
    out[i]  = dis[i] * ( sum_{e : col[e] == i} hn[row[e]]
                         + hn[i] )              # self-loop

Distribution: destination-node sharding.  Core k owns dest nodes
[k*SHARD, (k+1)*SHARD); every edge is routed to the core owning its dest.
Every core redundantly computes the full hn = dis * (x @ W) in bf16 (x
arrives transposed, bf16, slice-ROTATED per core so the core's own dest
slice is always slice 0 of its private layout; the dis premultiply is fused
into the psum->SBUF copy), writes hn to a DRAM scratch `h_perm` in a
permuted tile-major row layout with rows padded to 128 bf16 = 256B (the
dma_gather element granularity), then gathers its edges' source rows
edge-major with SWDGE dma_gather calls spread round-robin over the 4
hardware SWDGE queues and emitted several calls AHEAD of their consumers
(so pending consumer instructions never stall the dispatch of later
gathers), scatter-adds them into per-dest-block PSUM accumulators with
bf16 one-hot selection-matrix matmuls on the PE (sel[p, d] =
(col_local[slot p] == d), built by ONE broadcast tensor_tensor per gather
call), adds the self-loop term with a constant-identity matmul over the
SBUF-resident slice-0 hn rows, scales by dis[dest] with one broadcast
multiply per group, and writes its shard as one contiguous DMA.

The host does integer-only preprocessing: degree histogram, dis = deg**-0.5,
edge bucketing by (core, dest-block, source-chunk), fixed-size slot layout,
index/col packing, and the per-core rotated bf16 transpose of x.  The slot
layout is input-independent (fixed run length L_RUN per bucket), so the Bass
program is compiled once and cached; bucket overflow (a few hundred edges
for random graphs) is corrected exactly on the host.
"""

import math

import numpy as np

# ----------------------------------------------------------------------------
# configuration
# ----------------------------------------------------------------------------

N_NODES = 100000
DIM = 64
N_CORES = 8

P = 128  # partitions
HP = 128  # h_perm row width in bf16 (64 data + 64 pad -> 256B rows)


class Cfg:
    def __init__(self, n_nodes, dim, n_cores, slots_per_run,
                 blocks_per_group, chunk_slices=2, subcall_runs=None,
                 gbufs=6, selbufs=3):
        self.N = n_nodes
        self.DIM = dim
        self.NC = n_cores
        # 128-aligned dest shards; the last core's shard may be smaller
        self.NBLK = math.ceil(n_nodes / (n_cores * P))   # dest blocks / core
        self.SHARD = self.NBLK * P               # padded shard size
        self.SLICE = self.SHARD
        self.NS = n_cores
        self.row_base = np.arange(n_cores + 1) * self.SHARD
        self.H_ROWS = int(self.row_base[-1])
        assert self.SHARD * (n_cores - 1) < n_nodes <= self.H_ROWS
        self.CH_SL = chunk_slices                # slices per gather chunk
        self.NCH = math.ceil(self.NS / chunk_slices)
        self.crow = [int(self.row_base[min(c * chunk_slices, self.NS)])
                     for c in range(self.NCH + 1)]
        for c in range(self.NCH):
            assert self.crow[c + 1] - self.crow[c] <= 32767
        self.L_RUN = int(slots_per_run)          # slots per (block,chunk) run
        assert self.L_RUN % 64 == 0
        self.NBG = blocks_per_group              # dest blocks per psum group
        assert self.NBLK % blocks_per_group == 0
        self.NGRP = self.NBLK // blocks_per_group
        self.TOT = self.NBLK * self.NCH * self.L_RUN   # slots per core
        assert self.TOT % P == 0
        self.NTILES = self.TOT // P
        self.SR = subcall_runs or blocks_per_group  # runs per dma_gather call
        assert self.NBG % self.SR == 0
        assert (self.SR * self.L_RUN) % P == 0
        self.CALL_SLOTS = self.SR * self.L_RUN
        self.NCALLS = self.NGRP * self.NCH * (self.NBG // self.SR)
        self.IDXW = self.TOT // 16
        self.GBUFS = gbufs
        self.SELBUFS = selbufs

    def run_subtiles(self, g, c, b_):
        """K-subtiles of run (g, c, b_): [(abs_slot0, K), ...]."""
        out = []
        base = ((g * self.NCH + c) * self.NBG + b_) * self.L_RUN
        s = base
        end = s + self.L_RUN
        while s < end:
            if s % P:
                k = P - s % P
            else:
                k = min(P, end - s)
            out.append((s, k))
            s += k
        return out


CFG = Cfg(N_NODES, DIM, N_CORES, slots_per_run=640,
          blocks_per_group=14, subcall_runs=2, gbufs=16, selbufs=8)


def rho(cfg, n, core):
    """node id -> core-rotated permuted h_perm row (vectorized)."""
    s = n // cfg.SLICE
    m = n - s * cfg.SLICE
    sr = (s - core) % cfg.NS
    return sr * cfg.SHARD + (m % P) * cfg.NBLK + m // P


# ----------------------------------------------------------------------------
# host preprocessing
# ----------------------------------------------------------------------------

def host_prep(cfg, x, weight, edge_index):
    import ml_dtypes
    bf16 = ml_dtypes.bfloat16

    x = np.asarray(x, dtype=np.float32)
    weight = np.asarray(weight, dtype=np.float32)
    ei = np.asarray(edge_index)
    row = ei[0].astype(np.int64)
    col = ei[1].astype(np.int64)

    # deg counts outgoing (row) edges plus the implicit self-loop
    deg = (np.bincount(row, minlength=cfg.N) + 1).astype(np.float32)
    dis = deg ** -0.5

    k = np.minimum(col // cfg.SHARD, cfg.NC - 1)
    blk = (col % cfg.SHARD) // P
    col_local = (col % cfg.SHARD) % P
    g = blk // cfg.NBG
    b_ = blk % cfg.NBG
    s = row // cfg.SLICE
    run_in_core_base = (g * cfg.NCH) * cfg.NBG + b_  # chunk added per-core

    # per-core chunk and permuted row (rotation makes these core-dependent)
    idx_flat = np.zeros((cfg.NC, cfg.TOT), dtype=np.int16)
    colv = np.full((cfg.NC, cfg.TOT), -1.0, dtype=np.float32)
    ov_list = []
    crow_arr = np.asarray(cfg.crow)
    for core in range(cfg.NC):
        m = k == core
        rowm, colm = row[m], col[m]
        sr = (s[m] - core) % cfg.NS
        c = np.minimum(sr // cfg.CH_SL, cfg.NCH - 1)
        prow = rho(cfg, rowm, core)
        idxrel = prow - crow_arr[c]
        run = run_in_core_base[m] + c * cfg.NBG
        order = np.argsort(run, kind="stable")
        run_s = run[order]
        counts = np.bincount(run_s, minlength=cfg.NBLK * cfg.NCH)
        starts = np.concatenate([[0], np.cumsum(counts)])
        pos = np.arange(run_s.size) - starts[run_s]
        ok = pos < cfg.L_RUN
        slot = run_s * cfg.L_RUN + pos
        o = order[ok]
        idx_flat[core, slot[ok]] = idxrel[o].astype(np.int16)
        colv[core, slot[ok]] = col_local[m][o].astype(np.float32)
        ov_list.append((rowm[order[~ok]], colm[order[~ok]]))

    # colv packed tile-major [NC, 128, NTILES]
    colv_p = np.ascontiguousarray(
        colv.reshape(cfg.NC, cfg.NTILES, P).transpose(0, 2, 1))

    # per-call 16-partition wrapping of indices, replicated to 128 partitions
    idxw = idx_flat.reshape(cfg.NC, cfg.NCALLS, cfg.CALL_SLOTS // 16, 16)
    idxw = idxw.transpose(0, 3, 1, 2).reshape(cfg.NC, 16, cfg.IDXW)
    idxv_p = np.ascontiguousarray(np.tile(idxw, (1, 8, 1)))

    # dest-side dis [NC, 128, NBLK]
    disdest = np.ones((cfg.NC, cfg.NBLK * P), dtype=np.float32)
    ids = np.arange(cfg.SHARD)
    for core in range(cfg.NC):
        nd = min(cfg.SHARD, cfg.N - core * cfg.SHARD)
        disdest[core, :nd] = dis[core * cfg.SHARD + ids[:nd]]
    disdest = np.ascontiguousarray(
        disdest.reshape(cfg.NC, cfg.NBLK, P).transpose(0, 2, 1))

    # source-side dis in phase-1 layout [NC, 128, NS*NBLK]:
    # disn[core, p, s'*NBLK + j] = dis[node(core, s', j, p)]
    dis_pad = np.ones(cfg.H_ROWS, dtype=np.float32)
    # padded per-slice node table (original order)
    node_of = np.full((cfg.NS, cfg.SHARD), -1, dtype=np.int64)
    for sl in range(cfg.NS):
        n0 = sl * cfg.SLICE
        nn = min(cfg.SLICE, cfg.N - n0)
        node_of[sl, :nn] = np.arange(n0, n0 + nn)
    disn = np.ones((cfg.NC, P, cfg.NS * cfg.NBLK), dtype=np.float32)
    for core in range(cfg.NC):
        for sp in range(cfg.NS):
            sl = (sp + core) % cfg.NS
            nodes = node_of[sl].reshape(cfg.NBLK, P)  # [j, p]
            v = np.ones((cfg.NBLK, P), dtype=np.float32)
            valid = nodes >= 0
            v[valid] = dis[nodes[valid]]
            disn[core, :, sp * cfg.NBLK:(sp + 1) * cfg.NBLK] = v.T

    # rotated, zero-padded, bf16 x^T per core
    xt = np.ascontiguousarray(x.T)  # [DIM, N]
    xt_pad = np.zeros((cfg.DIM, cfg.H_ROWS), dtype=np.float32)
    for sl in range(cfg.NS):
        n0 = sl * cfg.SLICE
        nn = min(cfg.SLICE, cfg.N - n0)
        xt_pad[:, sl * cfg.SHARD:sl * cfg.SHARD + nn] = xt[:, n0:n0 + nn]
    xt_sl = xt_pad.reshape(cfg.DIM, cfg.NS, cfg.SHARD)

    iota = np.ascontiguousarray(
        np.broadcast_to(np.arange(P, dtype=np.float32), (P, P)))
    ident = np.eye(P, dtype=np.float32).astype(bf16)
    w_bf = weight.astype(bf16)

    in_maps = []
    for core in range(cfg.NC):
        rot = [(sp + core) % cfg.NS for sp in range(cfg.NS)]
        xtr = np.ascontiguousarray(
            xt_sl[:, rot, :].reshape(cfg.DIM, cfg.H_ROWS)).astype(bf16)
        in_maps.append({
            "xt": xtr,
            "w": w_bf,
            "iota": iota,
            "ident": ident,
            "colv": colv_p[core],
            "disn": disn[core],
            "disd": disdest[core],
            "idx": idxv_p[core],
        })

    # host correction for overflowed edges
    corr = None
    if any(r.size for r, c in ov_list):
        corr = np.zeros((cfg.N, cfg.DIM), dtype=np.float32)
        for r, cdst in ov_list:
            if not r.size:
                continue
            hsrc = (x[r].astype(bf16).astype(np.float32)
                    @ weight.astype(bf16).astype(np.float32))
            msk = hsrc * (dis[r] * dis[cdst])[:, None]
            np.add.at(corr, cdst, msk)
    return in_maps, corr


def unshard(cfg, outs, corr):
    out = np.empty((cfg.N, cfg.DIM), dtype=np.float32)
    for core in range(cfg.NC):
        o = outs[core]["outp"].reshape(P, cfg.NBLK, cfg.DIM)
        o = o.transpose(1, 0, 2).reshape(cfg.NBLK * P, cfg.DIM)
        nd = min(cfg.SHARD, cfg.N - core * cfg.SHARD)
        out[core * cfg.SHARD:core * cfg.SHARD + nd] = o[:nd]
    if corr is not None:
        out += corr
    return out


# ----------------------------------------------------------------------------
# device program
# ----------------------------------------------------------------------------

_PROG_CACHE = {}


def build_program(cfg, reps=1, phases="12"):
    import contextlib

    import concourse.bass as bass
    import concourse.tile as tile
    from concourse import bacc, mybir

    f32 = mybir.dt.float32
    bf16 = mybir.dt.bfloat16
    nc = bacc.Bacc("TRN2", target_bir_lowering=False, debug=False,
                   num_devices=cfg.NC, num_swdge_queues=4)

    J = cfg.NBLK
    xt = nc.dram_tensor("xt", [cfg.DIM, cfg.H_ROWS], bf16,
                        kind="ExternalInput")
    w = nc.dram_tensor("w", [cfg.DIM, cfg.DIM], bf16, kind="ExternalInput")
    iota = nc.dram_tensor("iota", [P, P], f32, kind="ExternalInput")
    ident = nc.dram_tensor("ident", [P, P], bf16, kind="ExternalInput")
    colv = nc.dram_tensor("colv", [P, cfg.NTILES], f32,
                          kind="ExternalInput")
    disn = nc.dram_tensor("disn", [P, cfg.NS * J], f32,
                          kind="ExternalInput")
    disd = nc.dram_tensor("disd", [P, J], f32, kind="ExternalInput")
    idx = nc.dram_tensor("idx", [P, cfg.IDXW], mybir.dt.int16,
                         kind="ExternalInput")
    outp = nc.dram_tensor("outp", [P, J * cfg.DIM], f32,
                          kind="ExternalOutput")
    h_perm = nc.dram_tensor("h_perm", [cfg.H_ROWS, HP], bf16)

    PSB = 8       # h blocks batched per psum bank in phase 1
    HS_HALF = 49  # blocks per h_perm write burst (half a slice)
    assert J == 2 * HS_HALF
    CW = cfg.CALL_SLOTS // 16   # idx columns per call
    CT = cfg.CALL_SLOTS // P    # slot tiles per call

    with tile.TileContext(nc) as tc:
      with (tc.For_i(0, reps, 1) if reps > 1 else contextlib.nullcontext()):
        with tc.tile_pool(name="cp", bufs=1) as cp, \
             tc.tile_pool(name="p1x", bufs=3) as xp, \
             tc.tile_pool(name="p1h", bufs=2) as hp, \
             tc.tile_pool(name="p1p", bufs=3, space="PSUM") as pp1, \
             tc.tile_pool(name="p2g", bufs=cfg.GBUFS) as gp, \
             tc.tile_pool(name="p2sel", bufs=cfg.SELBUFS) as selp, \
             tc.tile_pool(name="p2p", bufs=2, space="PSUM") as pp2:

            # ---------------- constants -----------------------------------
            w_sb = cp.tile([cfg.DIM, cfg.DIM], bf16)
            nc.sync.dma_start(out=w_sb[:], in_=w[:])
            iota_sb = cp.tile([P, P], f32)
            nc.sync.dma_start(out=iota_sb[:], in_=iota[:])
            ident_sb = cp.tile([P, P], bf16)
            nc.sync.dma_start(out=ident_sb[:], in_=ident[:])
            colv_sb = cp.tile([P, cfg.NTILES], f32)
            nc.sync.dma_start(out=colv_sb[:], in_=colv[:])
            disn_sb = cp.tile([P, cfg.NS * J], f32)
            nc.sync.dma_start(out=disn_sb[:], in_=disn[:])
            disd_sb = cp.tile([P, J], f32)
            nc.sync.dma_start(out=disd_sb[:], in_=disd[:])
            idx_sb = cp.tile([P, cfg.IDXW], mybir.dt.int16)
            nc.sync.dma_start(out=idx_sb[:], in_=idx[:])
            out_sb = cp.tile([P, J * cfg.DIM], f32)
            self_hs = cp.tile([P, J * HP], bf16)  # slice-0 hn, SBUF-resident

            # ---------------- phase 1: hn = dis * (x @ W), bf16 padded ----
            def ph1_slice(sp_):
                    for half in range(2):
                        if sp_ == 0:
                            hs = self_hs
                            hoff = half * HS_HALF * HP
                        else:
                            hs = hp.tile([P, HS_HALF * HP], bf16, tag="hs")
                            hoff = 0
                        for m in range(math.ceil(HS_HALF / PSB)):
                            j0 = half * HS_HALF + m * PSB
                            jn = min(PSB, HS_HALF - m * PSB)
                            xs = xp.tile([cfg.DIM, PSB * P], bf16, tag="xs")
                            nc.sync.dma_start(
                                out=xs[:, :jn * P],
                                in_=xt[:, sp_ * cfg.SHARD + j0 * P:
                                       sp_ * cfg.SHARD + (j0 + jn) * P])
                            ps = pp1.tile([P, PSB * cfg.DIM], f32)
                            for j8 in range(jn):
                                nc.tensor.matmul(
                                    out=ps[:, j8 * cfg.DIM:(j8 + 1) * cfg.DIM],
                                    lhsT=xs[:, j8 * P:(j8 + 1) * P],
                                    rhs=w_sb[:],
                                    start=True, stop=True)
                            # fused dis premultiply + f32->bf16 cast, into
                            # the low 64 of each 128-col padded block
                            dst = hs[:, hoff + m * PSB * HP:
                                     hoff + (m * PSB + jn) * HP]
                            dst = dst.rearrange("p (j d) -> p j d",
                                                d=HP)[:, :, :cfg.DIM]
                            nc.vector.tensor_tensor(
                                out=dst,
                                in0=ps[:, :jn * cfg.DIM].rearrange(
                                    "p (j d) -> p j d", d=cfg.DIM),
                                in1=disn_sb[:, sp_ * J + j0:
                                            sp_ * J + j0 + jn].unsqueeze(
                                    2).broadcast_to([P, jn, cfg.DIM]),
                                op=mybir.AluOpType.mult)
                        if sp_ == 0:
                            src = self_hs[:, half * HS_HALF * HP:
                                          (half + 1) * HS_HALF * HP]
                        else:
                            src = hs[:]
                        # rows sp_*SHARD + p*J + jj, jj in the half's range
                        dst = h_perm[sp_ * cfg.SHARD:(sp_ + 1) * cfg.SHARD, :]
                        dst = dst.rearrange("(p j) d -> p j d", p=P)
                        dst = dst[:, half * HS_HALF:(half + 1) * HS_HALF, :]
                        nc.sync.dma_start(
                            out=dst,
                            in_=src.rearrange("p (j d) -> p j d", d=HP))

            # ---------------- phase 2: gather + PE scatter-add -------------
            SPC = cfg.NBG // cfg.SR      # subcalls per (group, chunk)
            ncalls = cfg.NCALLS
            do_p2 = bool(set("2G") & set(phases))
            gather_only = do_p2 and "2" not in phases

            def emit_gather(j):
                c = (j // SPC) % cfg.NCH
                gb = gp.tile([P, CT, HP], bf16, tag="gbuf")
                nc.gpsimd.dma_gather(
                    out_ap=gb[:],
                    in_ap=h_perm[cfg.crow[c]:cfg.crow[c + 1], :],
                    idxs_ap=idx_sb[:, j * CW:(j + 1) * CW],
                    num_idxs=cfg.CALL_SLOTS,
                    num_idxs_reg=cfg.CALL_SLOTS,
                    elem_size=HP,
                    single_packet=False,
                    queue_num=j % 4,
                )
                return gb

            # emission order for phase overlap: phase-1 slices for chunk 0,
            # then upfront gathers interleaved with the remaining slices in
            # chunk-readiness order (a gather must be emitted AFTER the
            # h_perm writes it reads, or the framework sees WAR not RAW),
            # then the compute loop (whose dispatch stalls no longer gate
            # phase 1).
            gbufs = {}
            upfront = min(cfg.GBUFS, ncalls) if do_p2 else 0
            emitted = 0
            if "1" in phases:
                ph1_slice(0)
                ph1_slice(1)
            for c_ready in range(cfg.NCH):
                while (emitted < upfront
                       and (emitted // SPC) % cfg.NCH <= c_ready):
                    gbufs[emitted] = emit_gather(emitted)
                    emitted += 1
                if "1" in phases and c_ready < cfg.NCH - 1:
                    ph1_slice(2 * c_ready + 2)
                    ph1_slice(2 * c_ready + 3)
            while emitted < upfront:
                gbufs[emitted] = emit_gather(emitted)
                emitted += 1

            if do_p2:
                ps = None
                for j in range(ncalls):
                    sub = j % SPC
                    c = (j // SPC) % cfg.NCH
                    g = j // (SPC * cfg.NCH)
                    gb = gbufs.pop(j)
                    if j + cfg.GBUFS < ncalls:
                        gbufs[j + cfg.GBUFS] = emit_gather(j + cfg.GBUFS)
                    if gather_only:
                        nc.vector.tensor_copy(out=out_sb[:, :cfg.DIM],
                                              in_=gb[:, 0, :cfg.DIM])
                        continue
                    if c == 0 and sub == 0:
                        ps = pp2.tile([P, cfg.NBG * cfg.DIM], f32)
                    # one-hot selection matrices for all CT tiles of the call
                    sel = selp.tile([P, CT, P], bf16)
                    nc.vector.tensor_tensor(
                        out=sel[:],
                        in0=iota_sb[:].unsqueeze(1).broadcast_to([P, CT, P]),
                        in1=colv_sb[:, j * CT:(j + 1) * CT].unsqueeze(
                            2).broadcast_to([P, CT, P]),
                        op=mybir.AluOpType.is_equal)
                    for i in range(cfg.SR):
                        b_ = (j * cfg.SR + i) % cfg.NBG
                        for si, (s0, kk) in enumerate(
                                cfg.run_subtiles(g, c, b_)):
                            tloc = s0 // P - j * CT
                            p0 = s0 % P
                            nc.tensor.matmul(
                                out=ps[:, b_ * cfg.DIM:(b_ + 1) * cfg.DIM],
                                lhsT=sel[p0:p0 + kk, tloc, :],
                                rhs=gb[p0:p0 + kk, tloc, :cfg.DIM],
                                start=(c == 0 and si == 0 and b_ % 8 == 0),
                                stop=False, skip_group_check=True)
                    if c < cfg.NCH - 1 or sub < SPC - 1:
                        continue
                    # group tail: self-loop identity matmuls + dis[dest] scale
                    for b_ in range(cfg.NBG):
                        b = g * cfg.NBG + b_
                        nc.tensor.matmul(
                            out=ps[:, b_ * cfg.DIM:(b_ + 1) * cfg.DIM],
                            lhsT=ident_sb[:],
                            rhs=self_hs[:, b * HP:b * HP + cfg.DIM],
                            start=False, stop=True, skip_group_check=True)
                    g0 = g * cfg.NBG
                    nc.vector.tensor_tensor(
                        out=out_sb[:, g0 * cfg.DIM:
                                   (g0 + cfg.NBG) * cfg.DIM].rearrange(
                            "p (b d) -> p b d", d=cfg.DIM),
                        in0=ps[:].rearrange("p (b d) -> p b d", d=cfg.DIM),
                        in1=disd_sb[:, g0:g0 + cfg.NBG].unsqueeze(
                            2).broadcast_to([P, cfg.NBG, cfg.DIM]),
                        op=mybir.AluOpType.mult)
                if gather_only:
                    nc.vector.memset(out_sb[:, cfg.DIM:], 0)
                nc.sync.dma_start(out=outp[:], in_=out_sb[:])

    nc.compile()
    return nc


def build_with_queues(cfg, reps=1, phases="12", rotate=False):
    return build_program(cfg, reps=reps, phases=phases)


def get_program(cfg):
    key = (cfg.N, cfg.DIM, cfg.NC, cfg.SLICE, cfg.L_RUN, cfg.NBG,
           cfg.CH_SL, cfg.SR, cfg.GBUFS, cfg.SELBUFS)
    if key not in _PROG_CACHE:
        _PROG_CACHE[key] = build_program(cfg)
    return _PROG_CACHE[key]


# ----------------------------------------------------------------------------
# entry point
# ----------------------------------------------------------------------------

def kernel(x, weight, edge_index):
    from concourse.bass_utils import run_bass_kernel_spmd

    cfg = CFG
    in_maps, corr = host_prep(cfg, x, weight, edge_index)
    nc = get_program(cfg)
    res = run_bass_kernel_spmd(nc, in_maps, list(range(cfg.NC)))
    return unshard(cfg, res.results, corr)
